# revision 9
# baseline (speedup 1.0000x reference)
"""Trainium2 Bass kernel: discounted episode returns + normalization.

reference math (full [B, T] = [4096, 8192] f32 inputs):
    ret[t] = rew[t] + 0.99 * ret[t+1] * (1 - done[t])      (reverse-time scan)
    out = (ret - ret.mean()) / (ret.std(axis=-1, ddof=1, keepdims=True) + 1e-9)

Sharding: batch axis split across 8 NeuronCores (512 rows each). The scan is
data-parallel over batch; the global mean needs one scalar AllReduce.

v8 design notes (trace-driven, from v5/v6/v7):
- DVE runs the irreducible tensor_tensor_scan chain (~69us: 2.1 cyc/elem,
  no accelerated modes). Everything else is scheduled around keeping that
  chain stall-free and triggering the AllReduce immediately after it.
- ACT keeps a-coef + row-sum per unit (7.4us/unit < scan's 8.7us/unit).
  GpSimd must stay idle: it shares its SBUF port with the DVE, and v6
  measured a 40% scan slowdown when GpSimd streamed bulk work.
- a-coefs for the last 8 units (time chunks 0 and 1) are precomputed into
  dedicated tiles right after their (early-issued) done DMAs, so the scan
  tail never waits on ACT (v7 lost 9us there).
- Row sums for the final time chunk run on the DVE itself right after the
  last scans (v7 lost 11us waiting for ACT's backlog before the AR).
- Sum-of-squares passes are deferred into the AllReduce wait window
  (2 blocks on DVE scalar_tensor_tensor + accum, 2 on ACT Square + accum).
- Time chunks (512, 1024, 2048, 4096, 512 by position, processed in
  reverse): small first chunk warms the pipeline fast, small last chunk
  makes the AllReduce gate cheap.
- Two dummy warm-up AllReduces: at start (absorbs mesh cold-start) and
  mid-pipeline (v7 measured the real AR at 12.9us when another collective
  ran just before it, vs 22.4us cold).
- Output DMA per half block (8KB per-partition lines ~420GB/s; v7's
  quarter splits dropped to ~280GB/s).
- HBM traffic shrunk with narrow dtypes: rewards bf16 + done u8 in,
  output bf16 (upcast on the host). Scan state stays fp32, a-coefs exact
  fp32; only bf16 rounding of rewards/returns remains (~5e-3 vs 2e-2).
"""

from contextlib import ExitStack

import ml_dtypes
import numpy as np

import concourse.bass as bass
import concourse.mybir as mybir
import concourse.tile as tile
from concourse import bacc
from concourse.bass_utils import run_bass_kernel_spmd

F32 = mybir.dt.float32
BF16 = mybir.dt.bfloat16
U8 = mybir.dt.uint8
Alu = mybir.AluOpType
Act = mybir.ActivationFunctionType
AxL = mybir.AxisListType

DISCOUNT = 0.99
EPS = 1e-9
P = 128

N_CORES = 8
B_GLOBAL, T = 4096, 8192
B_CORE = B_GLOBAL // N_CORES
# time chunks by position; processed in reverse order
CHUNKS = (512, 1024, 2048, 4096, 512)
MAXCHUNK = max(CHUNKS)
N_EARLY = 2          # chunks 0..N_EARLY-1 get early done loads + a-coefs
SUM_DVE_CI = (0,)    # chunks whose row sums run on DVE right after the scan

WARMUP_AR = True
AR2_BEFORE_UNIT = 10  # fire warmup AR2 after this unit's row sum
N_SQ_DVE = 2


def _build_core_program(tc, out_ap, rew_ap, done_ap, n_cores, total_elems):
    nc = tc.nc
    B_core, T_ = rew_ap.shape
    n_blocks = B_core // P
    n_chunks = len(CHUNKS)
    starts = [sum(CHUNKS[:i]) for i in range(n_chunks)]
    assert sum(CHUNKS) == T_

    with ExitStack() as ctx:
        ret_pool = ctx.enter_context(tc.tile_pool(name="ret", bufs=1))
        rew_pool = ctx.enter_context(tc.tile_pool(name="rew", bufs=3))
        done_pool = ctx.enter_context(tc.tile_pool(name="done", bufs=3))
        a_pool = ctx.enter_context(tc.tile_pool(name="acoef", bufs=2))
        hold_pool = ctx.enter_context(tc.tile_pool(name="hold", bufs=1))
        stat_pool = ctx.enter_context(tc.tile_pool(name="stat", bufs=1))
        psum_pool = ctx.enter_context(tc.tile_pool(name="psum", bufs=1, space="PSUM"))
        dram_pool = ctx.enter_context(tc.tile_pool(name="dram", bufs=1, space="DRAM"))

        # don't-care outputs for the accum-bearing stats passes (only
        # accum_out matters). One per engine; each engine executes in order.
        act_scr = stat_pool.tile([P, MAXCHUNK], BF16, tag="act_scr",
                                 name="act_scr")
        dve_scr = stat_pool.tile([P, MAXCHUNK], BF16, tag="dve_scr",
                                 name="dve_scr")
        sum_cat = stat_pool.tile([P, n_blocks], F32)  # col b = row sums of block b
        ss_cat = stat_pool.tile([P, n_blocks], F32)   # col b = row sums of squares
        psum_s = psum_pool.tile([1, n_blocks], F32, tag="psum_s", name="psum_s")

        ret_tiles = []
        sum_part_tiles = []
        ss_part_tiles = []
        for b in range(n_blocks):
            ret_tiles.append(ret_pool.tile([P, T_], BF16, tag=f"ret{b}",
                                           name=f"ret{b}"))
            sum_part_tiles.append(stat_pool.tile([P, n_chunks], F32,
                                                 tag=f"smp{b}", name=f"smp{b}"))
            ss_part_tiles.append(stat_pool.tile([P, 2], F32,
                                                tag=f"ssp{b}", name=f"ssp{b}"))

        # stage the first chunk-row of loads before anything else (done
        # before rew: the a-coefficient chain starts from done)
        first_loads = []
        ci0 = n_chunks - 1
        lo0, hi0 = starts[ci0], starts[ci0] + CHUNKS[ci0]
        for b in range(n_blocks):
            rows = slice(b * P, (b + 1) * P)
            csz = CHUNKS[ci0]
            done_t = done_pool.tile([P, MAXCHUNK], U8, tag="done", name="done_t")
            nc.sync.dma_start(done_t[:, :csz], done_ap[rows, lo0:hi0])
            rew_t = rew_pool.tile([P, MAXCHUNK], BF16, tag="rew", name="rew_t")
            nc.sync.dma_start(rew_t[:, :csz], rew_ap[rows, lo0:hi0])
            first_loads.append((rew_t, done_t))

        # early done loads + a-coefs for the tail chunks (ci < N_EARLY):
        # dedicated tiles, computed while ACT is otherwise idle, so the
        # scan tail never waits on a late coefficient.
        a_hold = {}
        for ci in range(N_EARLY):
            csz = CHUNKS[ci]
            lo, hi = starts[ci], starts[ci] + csz
            for b in range(n_blocks):
                rows = slice(b * P, (b + 1) * P)
                dh = hold_pool.tile([P, csz], U8, tag=f"dh{ci}_{b}",
                                    name=f"dh{ci}_{b}")
                nc.sync.dma_start(dh[:], done_ap[rows, lo:hi])
                ah = hold_pool.tile([P, csz], F32, tag=f"ah{ci}_{b}",
                                    name=f"ah{ci}_{b}")
                nc.scalar.activation(ah[:], dh[:], Act.Copy,
                                     bias=DISCOUNT, scale=-DISCOUNT)
                a_hold[(ci, b)] = ah

        # warm-up AllReduce: absorbs the collective cold-start while the
        # compute engines stream the scan phase; nothing reads ar1_out
        if WARMUP_AR and n_cores > 1:
            z = stat_pool.tile([1, 1], F32, tag="z", name="z")
            nc.vector.memset(z[:], 0.0)
            ar1_in = dram_pool.tile([1, 1], F32, tag="ar1_in", name="ar1_in")
            ar1_out = dram_pool.tile([1, 1], F32, tag="ar1_out", name="ar1_out")
            nc.gpsimd.dma_start(ar1_in[:], z[:])
            nc.gpsimd.collective_compute(
                "AllReduce", Alu.add,
                replica_groups=[list(range(n_cores))],
                ins=[ar1_in.opt()], outs=[ar1_out.opt()])

        # main pipeline: reverse time order, interleaved across blocks so
        # back-to-back DVE scans are independent (the serial carry of a
        # block is n_blocks scans back)
        unit = 0
        for ci in range(n_chunks - 1, -1, -1):
            csz = CHUNKS[ci]
            lo, hi = starts[ci], starts[ci] + csz
            for b in range(n_blocks):
                rows = slice(b * P, (b + 1) * P)
                ret_t = ret_tiles[b]
                sum_parts = sum_part_tiles[b]
                if ci == n_chunks - 1:
                    rew_t, done_t = first_loads[b]
                else:
                    if ci >= N_EARLY:
                        done_t = done_pool.tile([P, MAXCHUNK], U8, tag="done",
                                                name="done_t")
                        nc.sync.dma_start(done_t[:, :csz], done_ap[rows, lo:hi])
                    rew_t = rew_pool.tile([P, MAXCHUNK], BF16, tag="rew",
                                          name="rew_t")
                    nc.sync.dma_start(rew_t[:, :csz], rew_ap[rows, lo:hi])
                # a = 0.99 - 0.99*done (exact fp32 coefficients). Unit 0 on
                # DVE (ACT's first op pays the activation-table load); tail
                # chunks precomputed above; the rest on ACT at priority 0.
                if ci < N_EARLY:
                    a_view = a_hold[(ci, b)][:]
                else:
                    a_t = a_pool.tile([P, MAXCHUNK], F32, tag="a", name="a_t")
                    a_view = a_t[:, :csz]
                    if unit == 0:
                        nc.vector.tensor_scalar(a_view, done_t[:, :csz],
                                                -DISCOUNT, DISCOUNT,
                                                Alu.mult, Alu.add)
                    else:
                        with tc.high_priority():
                            nc.scalar.activation(a_view, done_t[:, :csz],
                                                 Act.Copy, bias=DISCOUNT,
                                                 scale=-DISCOUNT)
                # reversed scan: state = a*state + rew, columns hi-1 .. lo
                init = 0.0 if ci == n_chunks - 1 else ret_t[:, hi:hi + 1]
                nc.vector.tensor_tensor_scan(
                    ret_t[:, lo:hi][:, ::-1], a_view[:, ::-1],
                    rew_t[:, :csz][:, ::-1],
                    init, Alu.mult, Alu.add)
                # row sums: final chunk on DVE (gates the AllReduce, and the
                # DVE is free right after its last scan); others on ACT.
                if ci in SUM_DVE_CI:
                    nc.vector.tensor_reduce(sum_parts[:, ci:ci + 1],
                                            ret_t[:, lo:hi], AxL.X, Alu.add)
                else:
                    nc.scalar.activation(act_scr[:, :csz], ret_t[:, lo:hi],
                                         Act.Copy,
                                         accum_out=sum_parts[:, ci:ci + 1])
                unit += 1
                # second warmup AR mid-pipeline: keeps the cc rings hot and
                # pre-aligns the cores for the real AllReduce.
                if WARMUP_AR and n_cores > 1 and unit == AR2_BEFORE_UNIT:
                    ar2_in = dram_pool.tile([1, 1], F32, tag="ar2_in",
                                            name="ar2_in")
                    ar2_out = dram_pool.tile([1, 1], F32, tag="ar2_out",
                                             name="ar2_out")
                    nc.gpsimd.dma_start(ar2_in[:], sum_parts[0:1, ci:ci + 1])
                    nc.gpsimd.collective_compute(
                        "AllReduce", Alu.add,
                        replica_groups=[list(range(n_cores))],
                        ins=[ar2_in.opt()], outs=[ar2_out.opt()])

        # ---- global-sum AllReduce critical path, before everything else ----
        ones_col = stat_pool.tile([P, 1], F32)
        nc.vector.memset(ones_col[:], 1.0)
        for b in range(n_blocks):
            nc.vector.tensor_reduce(sum_cat[:, b:b + 1], sum_part_tiles[b][:],
                                    AxL.X, Alu.add)
        nc.tensor.matmul(psum_s[:], ones_col[:], sum_cat[:], start=True, stop=True)
        s11 = stat_pool.tile([1, 1], F32)
        nc.vector.tensor_reduce(s11[:], psum_s[:], AxL.X, Alu.add)
        ones_row = stat_pool.tile([1, P], F32)
        nc.vector.memset(ones_row[:], 1.0)
        g_sb = stat_pool.tile([1, 1], F32)
        if n_cores > 1:
            ar_in = dram_pool.tile([1, 1], F32, tag="ar_in", name="ar_in")
            ar_out = dram_pool.tile([1, 1], F32, tag="ar_out", name="ar_out")
            nc.sync.dma_start(ar_in[:], s11[:])
            nc.gpsimd.collective_compute(
                "AllReduce", Alu.add,
                replica_groups=[list(range(n_cores))],
                ins=[ar_in.opt()], outs=[ar_out.opt()])
            nc.sync.dma_start(g_sb[:], ar_out[:])
        else:
            loc = dram_pool.tile([1, 1], F32, tag="loc", name="loc")
            nc.sync.dma_start(loc[:], s11[:])
            nc.sync.dma_start(g_sb[:], loc[:])

        # ---- sum-of-squares, two half-row passes per block, during the AR:
        # first N_SQ_DVE blocks on DVE (scalar_tensor_tensor + accum), the
        # rest on ACT (Square + accum).
        half = T_ // 2
        for b in range(n_blocks):
            ret_t = ret_tiles[b]
            ssp = ss_part_tiles[b]
            for h in range(2):
                cols = slice(h * half, (h + 1) * half)
                if b < N_SQ_DVE:
                    nc.vector.scalar_tensor_tensor(
                        dve_scr[:], ret_t[:, cols], 1.0, ret_t[:, cols],
                        Alu.mult, Alu.mult, accum_out=ssp[:, h:h + 1])
                else:
                    nc.scalar.activation(act_scr[:], ret_t[:, cols],
                                         Act.Square,
                                         accum_out=ssp[:, h:h + 1])
        for b in range(n_blocks):
            nc.vector.tensor_reduce(ss_cat[:, b:b + 1], ss_part_tiles[b][:],
                                    AxL.X, Alu.add)

        # ---- per-row 1/(std+eps): independent of the AllReduce ----
        sum_sq = stat_pool.tile([P, n_blocks], F32)
        nc.vector.tensor_tensor(sum_sq[:], sum_cat[:], sum_cat[:], Alu.mult)
        u = stat_pool.tile([P, n_blocks], F32)
        nc.vector.scalar_tensor_tensor(u[:], sum_sq[:], -1.0 / T_, ss_cat[:],
                                       Alu.mult, Alu.add)  # ss - sum^2/T
        stdv = stat_pool.tile([P, n_blocks], F32)
        nc.scalar.activation(stdv[:], u[:], Act.Sqrt, scale=1.0 / (T_ - 1))
        nc.vector.tensor_scalar_add(stdv[:], stdv[:], EPS)
        inv_cat = stat_pool.tile([P, n_blocks], F32)
        nc.vector.reciprocal(inv_cat[:], stdv[:])

        # The AR result comes back as a [1,1] DMA to partition 0, then a
        # ones[1,128] matmul replicates it across partitions in PSUM (~25ns)
        # -- cheaper than a 128-packet partition-broadcast DMA.
        psum_g = psum_pool.tile([P, 1], F32, tag="psum_g", name="psum_g")
        nc.tensor.matmul(psum_g[:], ones_row[:], g_sb[:], start=True, stop=True)

        negb_cat = stat_pool.tile([P, n_blocks], F32)
        nc.vector.tensor_scalar(negb_cat[:], inv_cat[:], psum_g[:, 0:1],
                                -1.0 / total_elems, Alu.mult, Alu.mult)

        # ---- normalize in place (bf16 4x mode), stream out per half block ----
        for b in range(n_blocks):
            rows = slice(b * P, (b + 1) * P)
            ret_t = ret_tiles[b]
            for h in range(2):
                cols = slice(h * half, (h + 1) * half)
                nc.vector.tensor_scalar(ret_t[:, cols], ret_t[:, cols],
                                        inv_cat[:, b:b + 1],
                                        negb_cat[:, b:b + 1],
                                        Alu.mult, Alu.add)
                nc.sync.dma_start(out_ap[rows, cols], ret_t[:, cols])


_NC_CACHE = None


def _get_nc():
    global _NC_CACHE
    if _NC_CACHE is None:
        nc = bacc.Bacc("TRN2", target_bir_lowering=False, debug=False,
                       enable_asserts=False, num_devices=N_CORES)
        rew = nc.dram_tensor("rewards", [B_CORE, T], BF16, kind="ExternalInput")
        done = nc.dram_tensor("done_flags", [B_CORE, T], U8, kind="ExternalInput")
        out = nc.dram_tensor("out", [B_CORE, T], BF16, kind="ExternalOutput")
        with tile.TileContext(nc) as tc:
            _build_core_program(tc, out.ap(), rew.ap(), done.ap(),
                                n_cores=N_CORES, total_elems=B_GLOBAL * T)
        nc.compile()
        _NC_CACHE = nc
    return _NC_CACHE


def run_sharded(rewards, done_flags, trace=False, **kwargs):
    """Run the SPMD kernel; returns (full_output, BassKernelResults)."""
    nc = _get_nc()
    rew16 = rewards.astype(ml_dtypes.bfloat16)
    done8 = done_flags.astype(np.uint8)
    in_maps = []
    for c in range(N_CORES):
        rows = slice(c * B_CORE, (c + 1) * B_CORE)
        in_maps.append({
            "rewards": np.ascontiguousarray(rew16[rows]),
            "done_flags": np.ascontiguousarray(done8[rows]),
        })
    res = run_bass_kernel_spmd(nc, in_maps, core_ids=list(range(N_CORES)),
                               trace=trace, **kwargs)
    full = np.concatenate(
        [res.results[c]["out"].astype(np.float32) for c in range(N_CORES)],
        axis=0)
    return full, res


def kernel(rewards, done_flags):
    out, _ = run_sharded(rewards, done_flags, trace=False)
    return out


# revision 13
# speedup vs baseline: 1.0232x; 1.0232x over previous
"""Trainium2 Bass kernel: discounted episode returns + normalization.

reference math (full [B, T] = [4096, 8192] f32 inputs):
    ret[t] = rew[t] + 0.99 * ret[t+1] * (1 - done[t])      (reverse-time scan)
    out = (ret - ret.mean()) / (ret.std(axis=-1, ddof=1, keepdims=True) + 1e-9)

Sharding: batch axis split across 8 NeuronCores (512 rows each). The scan is
data-parallel over batch; the global mean needs one scalar AllReduce.

v8 design notes (trace-driven, from v5/v6/v7):
- DVE runs the irreducible tensor_tensor_scan chain (~69us: 2.1 cyc/elem,
  no accelerated modes). Everything else is scheduled around keeping that
  chain stall-free and triggering the AllReduce immediately after it.
- ACT keeps a-coef + row-sum per unit (7.4us/unit < scan's 8.7us/unit).
  GpSimd must stay idle: it shares its SBUF port with the DVE, and v6
  measured a 40% scan slowdown when GpSimd streamed bulk work.
- a-coefs for the last 8 units (time chunks 0 and 1) are precomputed into
  dedicated tiles right after their (early-issued) done DMAs, so the scan
  tail never waits on ACT (v7 lost 9us there).
- Row sums for the final time chunk run on the DVE itself right after the
  last scans (v7 lost 11us waiting for ACT's backlog before the AR).
- Sum-of-squares passes are deferred into the AllReduce wait window
  (2 blocks on DVE scalar_tensor_tensor + accum, 2 on ACT Square + accum).
- Time chunks (512, 1024, 2048, 4096, 512 by position, processed in
  reverse): small first chunk warms the pipeline fast, small last chunk
  makes the AllReduce gate cheap.
- Two dummy warm-up AllReduces: at start (absorbs mesh cold-start) and
  mid-pipeline (v7 measured the real AR at 12.9us when another collective
  ran just before it, vs 22.4us cold).
- Output DMA per half block (8KB per-partition lines ~420GB/s; v7's
  quarter splits dropped to ~280GB/s).
- HBM traffic shrunk with narrow dtypes: rewards bf16 + done u8 in,
  output bf16 (upcast on the host). Scan state stays fp32, a-coefs exact
  fp32; only bf16 rounding of rewards/returns remains (~5e-3 vs 2e-2).
"""

from contextlib import ExitStack

import ml_dtypes
import numpy as np

import concourse.bass as bass
import concourse.mybir as mybir
import concourse.tile as tile
from concourse import bacc
from concourse.bass_utils import run_bass_kernel_spmd

F32 = mybir.dt.float32
BF16 = mybir.dt.bfloat16
U8 = mybir.dt.uint8
Alu = mybir.AluOpType
Act = mybir.ActivationFunctionType
AxL = mybir.AxisListType

DISCOUNT = 0.99
EPS = 1e-9
P = 128

N_CORES = 8
B_GLOBAL, T = 4096, 8192
B_CORE = B_GLOBAL // N_CORES
# time chunks by position; processed in reverse order
CHUNKS = (512, 1024, 2048, 4096, 512)
MAXCHUNK = max(CHUNKS)
A_HOLD_CI = (0,)     # chunks with early done loads + precomputed a-coefs
SUM_DVE_CI = (0,)    # chunks whose row sums run on DVE right after the scan

WARMUP_AR = True
AR2_BEFORE_UNIT = 9  # fire warmup AR2 after this many units' scans
N_SQ_DVE = 2


class _nullctx:
    def __enter__(self):
        return self

    def __exit__(self, *a):
        return False


def _build_core_program(tc, out_ap, rew_ap, done_ap, n_cores, total_elems):
    nc = tc.nc
    B_core, T_ = rew_ap.shape
    n_blocks = B_core // P
    n_chunks = len(CHUNKS)
    starts = [sum(CHUNKS[:i]) for i in range(n_chunks)]
    assert sum(CHUNKS) == T_

    with ExitStack() as ctx:
        ret_pool = ctx.enter_context(tc.tile_pool(name="ret", bufs=1))
        rew_pool = ctx.enter_context(tc.tile_pool(name="rew", bufs=3))
        done_pool = ctx.enter_context(tc.tile_pool(name="done", bufs=3))
        a_pool = ctx.enter_context(tc.tile_pool(name="acoef", bufs=2))
        hold_pool = ctx.enter_context(tc.tile_pool(name="hold", bufs=1))
        stat_pool = ctx.enter_context(tc.tile_pool(name="stat", bufs=1))
        psum_pool = ctx.enter_context(tc.tile_pool(name="psum", bufs=1, space="PSUM"))
        dram_pool = ctx.enter_context(tc.tile_pool(name="dram", bufs=1, space="DRAM"))

        # don't-care outputs for the accum-bearing stats passes (only
        # accum_out matters). One per engine; each engine executes in order.
        act_scr = stat_pool.tile([P, MAXCHUNK], BF16, tag="act_scr",
                                 name="act_scr")
        dve_scr = stat_pool.tile([P, MAXCHUNK], BF16, tag="dve_scr",
                                 name="dve_scr")
        sum_cat = stat_pool.tile([P, n_blocks], F32)  # col b = row sums of block b
        ss_cat = stat_pool.tile([P, n_blocks], F32)   # col b = row sums of squares
        psum_s = psum_pool.tile([1, n_blocks], F32, tag="psum_s", name="psum_s")

        ret_tiles = []
        sum_part_tiles = []
        ss_part_tiles = []
        for b in range(n_blocks):
            ret_tiles.append(ret_pool.tile([P, T_], BF16, tag=f"ret{b}",
                                           name=f"ret{b}"))
            sum_part_tiles.append(stat_pool.tile([P, n_chunks], F32,
                                                 tag=f"smp{b}", name=f"smp{b}"))
            ss_part_tiles.append(stat_pool.tile([P, 2], F32,
                                                tag=f"ssp{b}", name=f"ssp{b}"))

        # stage the first chunk-row of loads before anything else (done
        # before rew: the a-coefficient chain starts from done)
        first_loads = []
        ci0 = n_chunks - 1
        lo0, hi0 = starts[ci0], starts[ci0] + CHUNKS[ci0]
        for b in range(n_blocks):
            rows = slice(b * P, (b + 1) * P)
            csz = CHUNKS[ci0]
            done_t = done_pool.tile([P, MAXCHUNK], U8, tag="done", name="done_t")
            nc.sync.dma_start(done_t[:, :csz], done_ap[rows, lo0:hi0])
            rew_t = rew_pool.tile([P, MAXCHUNK], BF16, tag="rew", name="rew_t")
            nc.sync.dma_start(rew_t[:, :csz], rew_ap[rows, lo0:hi0])
            first_loads.append((rew_t, done_t))

        a_hold = {}

        # warm-up AllReduce: absorbs the collective cold-start while the
        # compute engines stream the scan phase; nothing reads ar1_out
        if WARMUP_AR and n_cores > 1:
            z = stat_pool.tile([1, 1], F32, tag="z", name="z")
            nc.vector.memset(z[:], 0.0)
            ar1_in = dram_pool.tile([1, 1], F32, tag="ar1_in", name="ar1_in")
            ar1_out = dram_pool.tile([1, 1], F32, tag="ar1_out", name="ar1_out")
            nc.gpsimd.dma_start(ar1_in[:], z[:])
            nc.gpsimd.collective_compute(
                "AllReduce", Alu.add,
                replica_groups=[list(range(n_cores))],
                ins=[ar1_in.opt()], outs=[ar1_out.opt()])

        # main pipeline: reverse time order, interleaved across blocks so
        # back-to-back DVE scans are independent (the serial carry of a
        # block is n_blocks scans back)
        unit = 0
        for ci in range(n_chunks - 1, -1, -1):
            csz = CHUNKS[ci]
            lo, hi = starts[ci], starts[ci] + csz
            for b in range(n_blocks):
                rows = slice(b * P, (b + 1) * P)
                ret_t = ret_tiles[b]
                sum_parts = sum_part_tiles[b]
                if ci == n_chunks - 1:
                    rew_t, done_t = first_loads[b]
                else:
                    if ci not in A_HOLD_CI:
                        done_t = done_pool.tile([P, MAXCHUNK], U8, tag="done",
                                                name="done_t")
                        nc.sync.dma_start(done_t[:, :csz], done_ap[rows, lo:hi])
                    rew_t = rew_pool.tile([P, MAXCHUNK], BF16, tag="rew",
                                          name="rew_t")
                    nc.sync.dma_start(rew_t[:, :csz], rew_ap[rows, lo:hi])
                # a = 0.99 - 0.99*done (exact fp32 coefficients). Unit 0 on
                # DVE (ACT's first op pays the activation-table load); tail
                # chunks precomputed mid-loop; the rest on ACT at priority 0.
                if ci in A_HOLD_CI:
                    a_view = a_hold[(ci, b)][:]
                else:
                    a_t = a_pool.tile([P, MAXCHUNK], F32, tag="a", name="a_t")
                    a_view = a_t[:, :csz]
                    if unit == 0:
                        nc.vector.tensor_scalar(a_view, done_t[:, :csz],
                                                -DISCOUNT, DISCOUNT,
                                                Alu.mult, Alu.add)
                    else:
                        with tc.high_priority():
                            nc.scalar.activation(a_view, done_t[:, :csz],
                                                 Act.Copy, bias=DISCOUNT,
                                                 scale=-DISCOUNT)
                # reversed scan: state = a*state + rew, columns hi-1 .. lo
                init = 0.0 if ci == n_chunks - 1 else ret_t[:, hi:hi + 1]
                scan_hp = tc.high_priority() if ci in SUM_DVE_CI else _nullctx()
                with scan_hp:
                    nc.vector.tensor_tensor_scan(
                        ret_t[:, lo:hi][:, ::-1], a_view[:, ::-1],
                        rew_t[:, :csz][:, ::-1],
                        init, Alu.mult, Alu.add)
                    # row sums: final chunk on DVE (gates the AllReduce, and
                    # the DVE is free right after its last scan); others ACT.
                    if ci in SUM_DVE_CI:
                        nc.vector.tensor_reduce(sum_parts[:, ci:ci + 1],
                                                ret_t[:, lo:hi], AxL.X,
                                                Alu.add)
                if ci not in SUM_DVE_CI:
                    nc.scalar.activation(act_scr[:, :csz], ret_t[:, lo:hi],
                                         Act.Copy,
                                         accum_out=sum_parts[:, ci:ci + 1])
                unit += 1
                # second warmup AR mid-pipeline (wired to this unit's scan
                # output, not an ACT product): keeps the cc rings hot and
                # pre-aligns the cores for the real AllReduce.
                if WARMUP_AR and n_cores > 1 and unit == AR2_BEFORE_UNIT:
                    ar2_in = dram_pool.tile([1, 1], F32, tag="ar2_in",
                                            name="ar2_in")
                    ar2_out = dram_pool.tile([1, 1], F32, tag="ar2_out",
                                             name="ar2_out")
                    nc.gpsimd.dma_start(ar2_in[:], ret_t[0:1, lo:lo + 1])
                    nc.gpsimd.collective_compute(
                        "AllReduce", Alu.add,
                        replica_groups=[list(range(n_cores))],
                        ins=[ar2_in.opt()], outs=[ar2_out.opt()])
            # after the big chunk's loads are all issued: early done loads +
            # a-coefs for the held tail chunks, on otherwise-idle ACT time
            if ci == n_chunks - 2:
                for hci in A_HOLD_CI:
                    hsz = CHUNKS[hci]
                    hlo = starts[hci]
                    for hb in range(n_blocks):
                        hrows = slice(hb * P, (hb + 1) * P)
                        dh = hold_pool.tile([P, hsz], U8, tag=f"dh{hci}_{hb}",
                                            name=f"dh{hci}_{hb}")
                        nc.sync.dma_start(dh[:], done_ap[hrows, hlo:hlo + hsz])
                        ah = hold_pool.tile([P, hsz], F32, tag=f"ah{hci}_{hb}",
                                            name=f"ah{hci}_{hb}")
                        nc.scalar.activation(ah[:], dh[:], Act.Copy,
                                             bias=DISCOUNT, scale=-DISCOUNT)
                        a_hold[(hci, hb)] = ah

        # ---- global-sum AllReduce critical path, before everything else ----
        with tc.high_priority():
            ones_col = stat_pool.tile([P, 1], F32)
            nc.vector.memset(ones_col[:], 1.0)
            for b in range(n_blocks):
                nc.vector.tensor_reduce(sum_cat[:, b:b + 1],
                                        sum_part_tiles[b][:], AxL.X, Alu.add)
            nc.tensor.matmul(psum_s[:], ones_col[:], sum_cat[:], start=True,
                             stop=True)
            s11 = stat_pool.tile([1, 1], F32)
            nc.vector.tensor_reduce(s11[:], psum_s[:], AxL.X, Alu.add)
            ones_row = stat_pool.tile([1, P], F32)
            nc.vector.memset(ones_row[:], 1.0)
            g_sb = stat_pool.tile([1, 1], F32)
            if n_cores > 1:
                ar_in = dram_pool.tile([1, 1], F32, tag="ar_in", name="ar_in")
                ar_out = dram_pool.tile([1, 1], F32, tag="ar_out",
                                        name="ar_out")
                nc.sync.dma_start(ar_in[:], s11[:])
                nc.gpsimd.collective_compute(
                    "AllReduce", Alu.add,
                    replica_groups=[list(range(n_cores))],
                    ins=[ar_in.opt()], outs=[ar_out.opt()])
                nc.sync.dma_start(g_sb[:], ar_out[:])
            else:
                loc = dram_pool.tile([1, 1], F32, tag="loc", name="loc")
                nc.sync.dma_start(loc[:], s11[:])
                nc.sync.dma_start(g_sb[:], loc[:])

        # ---- sum-of-squares, two half-row passes per block, during the AR:
        # first N_SQ_DVE blocks on DVE (scalar_tensor_tensor + accum), the
        # rest on ACT (Square + accum).
        half = T_ // 2
        for b in range(n_blocks):
            ret_t = ret_tiles[b]
            ssp = ss_part_tiles[b]
            for h in range(2):
                cols = slice(h * half, (h + 1) * half)
                if b < N_SQ_DVE:
                    nc.vector.scalar_tensor_tensor(
                        dve_scr[:], ret_t[:, cols], 1.0, ret_t[:, cols],
                        Alu.mult, Alu.mult, accum_out=ssp[:, h:h + 1])
                else:
                    nc.scalar.activation(act_scr[:], ret_t[:, cols],
                                         Act.Square,
                                         accum_out=ssp[:, h:h + 1])
        for b in range(n_blocks):
            nc.vector.tensor_reduce(ss_cat[:, b:b + 1], ss_part_tiles[b][:],
                                    AxL.X, Alu.add)

        # ---- per-row 1/(std+eps): independent of the AllReduce ----
        sum_sq = stat_pool.tile([P, n_blocks], F32)
        nc.vector.tensor_tensor(sum_sq[:], sum_cat[:], sum_cat[:], Alu.mult)
        u = stat_pool.tile([P, n_blocks], F32)
        nc.vector.scalar_tensor_tensor(u[:], sum_sq[:], -1.0 / T_, ss_cat[:],
                                       Alu.mult, Alu.add)  # ss - sum^2/T
        stdv = stat_pool.tile([P, n_blocks], F32)
        nc.scalar.activation(stdv[:], u[:], Act.Sqrt, scale=1.0 / (T_ - 1))
        nc.vector.tensor_scalar_add(stdv[:], stdv[:], EPS)
        inv_cat = stat_pool.tile([P, n_blocks], F32)
        nc.vector.reciprocal(inv_cat[:], stdv[:])

        # The AR result comes back as a [1,1] DMA to partition 0, then a
        # ones[1,128] matmul replicates it across partitions in PSUM (~25ns)
        # -- cheaper than a 128-packet partition-broadcast DMA.
        psum_g = psum_pool.tile([P, 1], F32, tag="psum_g", name="psum_g")
        nc.tensor.matmul(psum_g[:], ones_row[:], g_sb[:], start=True, stop=True)

        negb_cat = stat_pool.tile([P, n_blocks], F32)
        nc.vector.tensor_scalar(negb_cat[:], inv_cat[:], psum_g[:, 0:1],
                                -1.0 / total_elems, Alu.mult, Alu.mult)

        # ---- normalize in place (bf16 4x mode), stream out per half block ----
        for b in range(n_blocks):
            rows = slice(b * P, (b + 1) * P)
            ret_t = ret_tiles[b]
            for h in range(2):
                cols = slice(h * half, (h + 1) * half)
                nc.vector.tensor_scalar(ret_t[:, cols], ret_t[:, cols],
                                        inv_cat[:, b:b + 1],
                                        negb_cat[:, b:b + 1],
                                        Alu.mult, Alu.add)
                nc.sync.dma_start(out_ap[rows, cols], ret_t[:, cols])


_NC_CACHE = None


def _get_nc():
    global _NC_CACHE
    if _NC_CACHE is None:
        nc = bacc.Bacc("TRN2", target_bir_lowering=False, debug=False,
                       enable_asserts=False, num_devices=N_CORES)
        rew = nc.dram_tensor("rewards", [B_CORE, T], BF16, kind="ExternalInput")
        done = nc.dram_tensor("done_flags", [B_CORE, T], U8, kind="ExternalInput")
        out = nc.dram_tensor("out", [B_CORE, T], BF16, kind="ExternalOutput")
        with tile.TileContext(nc) as tc:
            _build_core_program(tc, out.ap(), rew.ap(), done.ap(),
                                n_cores=N_CORES, total_elems=B_GLOBAL * T)
        nc.compile()
        _NC_CACHE = nc
    return _NC_CACHE


def run_sharded(rewards, done_flags, trace=False, **kwargs):
    """Run the SPMD kernel; returns (full_output, BassKernelResults)."""
    nc = _get_nc()
    rew16 = rewards.astype(ml_dtypes.bfloat16)
    done8 = done_flags.astype(np.uint8)
    in_maps = []
    for c in range(N_CORES):
        rows = slice(c * B_CORE, (c + 1) * B_CORE)
        in_maps.append({
            "rewards": np.ascontiguousarray(rew16[rows]),
            "done_flags": np.ascontiguousarray(done8[rows]),
        })
    res = run_bass_kernel_spmd(nc, in_maps, core_ids=list(range(N_CORES)),
                               trace=trace, **kwargs)
    full = np.concatenate(
        [res.results[c]["out"].astype(np.float32) for c in range(N_CORES)],
        axis=0)
    return full, res


def kernel(rewards, done_flags):
    out, _ = run_sharded(rewards, done_flags, trace=False)
    return out


# revision 14
# speedup vs baseline: 1.0771x; 1.0527x over previous
"""Trainium2 Bass kernel: discounted episode returns + normalization.

reference math (full [B, T] = [4096, 8192] f32 inputs):
    ret[t] = rew[t] + 0.99 * ret[t+1] * (1 - done[t])      (reverse-time scan)
    out = (ret - ret.mean()) / (ret.std(axis=-1, ddof=1, keepdims=True) + 1e-9)

Sharding: batch axis split across 8 NeuronCores (512 rows each). The scan is
data-parallel over batch; the global mean needs one scalar AllReduce.

v10 design notes (trace-driven; see git-style history in earlier versions):
- DVE runs the irreducible tensor_tensor_scan chain (~69us: 2.1 cyc/elem,
  fp32-state, no accelerated modes). Everything else is scheduled around
  keeping that chain stall-free and triggering the AllReduce immediately
  after it. GpSimd stays idle: it shares its SBUF port with the DVE (v6
  measured a 40% scan slowdown when it streamed bulk work).
- Time chunks (512, 1024, 2048, 2048, 2048, 512 by position, processed in
  reverse). 2048 pool tiles let the input DMA prefetch 4 units deep in
  little SBUF; v9's scan tail stalled ~4us waiting for the last rew DMA
  behind a 3-deep MAXCHUNK=4096 pool.
- The last two chunk waves (time chunks 0 and 1) use dedicated hold tiles
  for done, rew AND the a-coefficients, loaded/computed mid-pipeline, so
  the scan tail depends on nothing issued late.
- The AllReduce-trigger chain is ACT+PE only (activation Copy accum_out
  reductions -> ones-matmul -> PSUM -> SBUF copy -> DMA): v9 put it on the
  DVE, where the scheduler slotted 4.4us sum-of-squares passes into its
  bubbles and delayed the AR trigger ~8us.
- Sum-of-squares is deferred into the AllReduce wait window (blocks 0-1 on
  DVE via scalar_tensor_tensor accum, blocks 2-3 on ACT Square accum).
- Two dummy warm-up AllReduces (start + mid-pipeline) keep the collective
  rings warm; measured real-AR duration still varies 13-27us (inter-core
  arrival skew) -- that variance is outside the kernel's control.
- Output DMA per half block (8KB per-partition lines; quarter splits
  measured ~280GB/s vs ~420GB/s for halves).
- HBM traffic shrunk with narrow dtypes: rewards bf16 + done u8 in,
  output bf16 (upcast on the host). Scan state stays fp32, a-coefs exact
  fp32; only bf16 rounding of rewards/returns remains (~5.6e-3 vs 2e-2).
"""

from contextlib import ExitStack

import ml_dtypes
import numpy as np

import concourse.bass as bass
import concourse.mybir as mybir
import concourse.tile as tile
from concourse import bacc
from concourse.bass_utils import run_bass_kernel_spmd

F32 = mybir.dt.float32
BF16 = mybir.dt.bfloat16
U8 = mybir.dt.uint8
Alu = mybir.AluOpType
Act = mybir.ActivationFunctionType
AxL = mybir.AxisListType

DISCOUNT = 0.99
EPS = 1e-9
P = 128

N_CORES = 8
B_GLOBAL, T = 4096, 8192
B_CORE = B_GLOBAL // N_CORES
# time chunks by position; processed in reverse order
CHUNKS = (512, 1024, 2048, 2048, 2048, 512)
MAXCHUNK = max(CHUNKS)
A_HOLD_CI = (0, 1)   # chunks with held done/rew loads + precomputed a-coefs

WARMUP_AR = True
AR2_BEFORE_UNIT = 16  # fire warmup AR2 after this many units' scans
N_SQ_DVE = 2


def _build_core_program(tc, out_ap, rew_ap, done_ap, n_cores, total_elems):
    nc = tc.nc
    B_core, T_ = rew_ap.shape
    n_blocks = B_core // P
    n_chunks = len(CHUNKS)
    starts = [sum(CHUNKS[:i]) for i in range(n_chunks)]
    assert sum(CHUNKS) == T_

    with ExitStack() as ctx:
        ret_pool = ctx.enter_context(tc.tile_pool(name="ret", bufs=1))
        rew_pool = ctx.enter_context(tc.tile_pool(name="rew", bufs=4))
        done_pool = ctx.enter_context(tc.tile_pool(name="done", bufs=4))
        a_pool = ctx.enter_context(tc.tile_pool(name="acoef", bufs=3))
        hold_pool = ctx.enter_context(tc.tile_pool(name="hold", bufs=1))
        stat_pool = ctx.enter_context(tc.tile_pool(name="stat", bufs=1))
        psum_pool = ctx.enter_context(tc.tile_pool(name="psum", bufs=1, space="PSUM"))
        dram_pool = ctx.enter_context(tc.tile_pool(name="dram", bufs=1, space="DRAM"))

        # don't-care outputs for the accum-bearing stats passes (only
        # accum_out matters). One per engine; each engine executes in order.
        act_scr = stat_pool.tile([P, MAXCHUNK], BF16, tag="act_scr",
                                 name="act_scr")
        dve_scr = stat_pool.tile([P, MAXCHUNK], BF16, tag="dve_scr",
                                 name="dve_scr")
        sum_cat = stat_pool.tile([P, n_blocks], F32)  # col b = row sums of block b
        ss_cat = stat_pool.tile([P, n_blocks], F32)   # col b = row sums of squares
        colsum = stat_pool.tile([P, 1], F32)          # per-partition core total

        ret_tiles = []
        sum_part_tiles = []
        ss_part_tiles = []
        for b in range(n_blocks):
            ret_tiles.append(ret_pool.tile([P, T_], BF16, tag=f"ret{b}",
                                           name=f"ret{b}"))
            sum_part_tiles.append(stat_pool.tile([P, n_chunks], F32,
                                                 tag=f"smp{b}", name=f"smp{b}"))
            ss_part_tiles.append(stat_pool.tile([P, 4], F32,
                                                tag=f"ssp{b}", name=f"ssp{b}"))

        # stage the first chunk-row of loads before anything else (done
        # before rew: the a-coefficient chain starts from done)
        first_loads = []
        ci0 = n_chunks - 1
        lo0, hi0 = starts[ci0], starts[ci0] + CHUNKS[ci0]
        for b in range(n_blocks):
            rows = slice(b * P, (b + 1) * P)
            csz = CHUNKS[ci0]
            done_t = done_pool.tile([P, MAXCHUNK], U8, tag="done", name="done_t")
            nc.sync.dma_start(done_t[:, :csz], done_ap[rows, lo0:hi0])
            rew_t = rew_pool.tile([P, MAXCHUNK], BF16, tag="rew", name="rew_t")
            nc.sync.dma_start(rew_t[:, :csz], rew_ap[rows, lo0:hi0])
            first_loads.append((rew_t, done_t))

        a_hold = {}
        rew_hold = {}

        # warm-up AllReduce: absorbs the collective cold-start while the
        # compute engines stream the scan phase; nothing reads ar1_out
        if WARMUP_AR and n_cores > 1:
            z = stat_pool.tile([1, 1], F32, tag="z", name="z")
            nc.vector.memset(z[:], 0.0)
            ar1_in = dram_pool.tile([1, 1], F32, tag="ar1_in", name="ar1_in")
            ar1_out = dram_pool.tile([1, 1], F32, tag="ar1_out", name="ar1_out")
            nc.gpsimd.dma_start(ar1_in[:], z[:])
            nc.gpsimd.collective_compute(
                "AllReduce", Alu.add,
                replica_groups=[list(range(n_cores))],
                ins=[ar1_in.opt()], outs=[ar1_out.opt()])

        # main pipeline: reverse time order, interleaved across blocks so
        # back-to-back DVE scans are independent (the serial carry of a
        # block is n_blocks scans back)
        unit = 0
        for ci in range(n_chunks - 1, -1, -1):
            csz = CHUNKS[ci]
            lo, hi = starts[ci], starts[ci] + csz
            for b in range(n_blocks):
                rows = slice(b * P, (b + 1) * P)
                ret_t = ret_tiles[b]
                sum_parts = sum_part_tiles[b]
                if ci == n_chunks - 1:
                    rew_t, done_t = first_loads[b]
                    rew_view = rew_t[:, :csz]
                elif ci in A_HOLD_CI:
                    rew_view = rew_hold[(ci, b)][:]
                else:
                    done_t = done_pool.tile([P, MAXCHUNK], U8, tag="done",
                                            name="done_t")
                    nc.sync.dma_start(done_t[:, :csz], done_ap[rows, lo:hi])
                    rew_t = rew_pool.tile([P, MAXCHUNK], BF16, tag="rew",
                                          name="rew_t")
                    nc.sync.dma_start(rew_t[:, :csz], rew_ap[rows, lo:hi])
                    rew_view = rew_t[:, :csz]
                # a = 0.99 - 0.99*done (exact fp32 coefficients). Unit 0 on
                # DVE (ACT's first op pays the activation-table load); tail
                # chunks precomputed mid-loop; the rest on ACT at priority 0.
                if ci in A_HOLD_CI:
                    a_view = a_hold[(ci, b)][:]
                else:
                    a_t = a_pool.tile([P, MAXCHUNK], F32, tag="a", name="a_t")
                    a_view = a_t[:, :csz]
                    if unit == 0:
                        nc.vector.tensor_scalar(a_view, done_t[:, :csz],
                                                -DISCOUNT, DISCOUNT,
                                                Alu.mult, Alu.add)
                    else:
                        with tc.high_priority():
                            nc.scalar.activation(a_view, done_t[:, :csz],
                                                 Act.Copy, bias=DISCOUNT,
                                                 scale=-DISCOUNT)
                # reversed scan: state = a*state + rew, columns hi-1 .. lo
                init = 0.0 if ci == n_chunks - 1 else ret_t[:, hi:hi + 1]
                nc.vector.tensor_tensor_scan(
                    ret_t[:, lo:hi][:, ::-1], a_view[:, ::-1],
                    rew_view[:, ::-1],
                    init, Alu.mult, Alu.add)
                # row sums on ACT (Copy + accum_out): they feed the AllReduce
                nc.scalar.activation(act_scr[:, :csz], ret_t[:, lo:hi],
                                     Act.Copy,
                                     accum_out=sum_parts[:, ci:ci + 1])
                unit += 1
                # second warmup AR mid-pipeline (wired to this unit's scan
                # output): keeps the cc rings hot for the real AllReduce.
                if WARMUP_AR and n_cores > 1 and unit == AR2_BEFORE_UNIT:
                    ar2_in = dram_pool.tile([1, 1], F32, tag="ar2_in",
                                            name="ar2_in")
                    ar2_out = dram_pool.tile([1, 1], F32, tag="ar2_out",
                                             name="ar2_out")
                    nc.gpsimd.dma_start(ar2_in[:], ret_t[0:1, lo:lo + 1])
                    nc.gpsimd.collective_compute(
                        "AllReduce", Alu.add,
                        replica_groups=[list(range(n_cores))],
                        ins=[ar2_in.opt()], outs=[ar2_out.opt()])
            # after the first big wave's loads are all issued: held loads +
            # a-coefs for the tail chunks, on otherwise-idle ACT/DMA time
            if ci == n_chunks - 2:
                for hci in A_HOLD_CI:
                    hsz = CHUNKS[hci]
                    hlo = starts[hci]
                    for hb in range(n_blocks):
                        hrows = slice(hb * P, (hb + 1) * P)
                        dh = hold_pool.tile([P, hsz], U8, tag=f"dh{hci}_{hb}",
                                            name=f"dh{hci}_{hb}")
                        nc.sync.dma_start(dh[:], done_ap[hrows, hlo:hlo + hsz])
                        rh = hold_pool.tile([P, hsz], BF16,
                                            tag=f"rh{hci}_{hb}",
                                            name=f"rh{hci}_{hb}")
                        nc.sync.dma_start(rh[:], rew_ap[hrows, hlo:hlo + hsz])
                        rew_hold[(hci, hb)] = rh
                        ah = hold_pool.tile([P, hsz], F32, tag=f"ah{hci}_{hb}",
                                            name=f"ah{hci}_{hb}")
                        nc.scalar.activation(ah[:], dh[:], Act.Copy,
                                             bias=DISCOUNT, scale=-DISCOUNT)
                        a_hold[(hci, hb)] = ah

        # ---- global-sum AllReduce critical path: ACT + PE only, so the
        # DVE's deferred sum-of-squares passes cannot delay it.
        ones_col = stat_pool.tile([P, 1], F32)
        nc.vector.memset(ones_col[:], 1.0)
        psum_t = psum_pool.tile([1, 1], F32, tag="psum_t", name="psum_t")
        with tc.high_priority():
            for b in range(n_blocks):
                nc.scalar.activation(act_scr[:, :n_chunks],
                                     sum_part_tiles[b][:], Act.Copy,
                                     accum_out=sum_cat[:, b:b + 1])
            nc.scalar.activation(act_scr[:, :n_blocks], sum_cat[:], Act.Copy,
                                 accum_out=colsum[:])
            nc.tensor.matmul(psum_t[:], ones_col[:], colsum[:], start=True,
                             stop=True)
            s11 = stat_pool.tile([1, 1], F32)
            nc.scalar.activation(s11[:], psum_t[:], Act.Copy)
            g_sb = stat_pool.tile([1, 1], F32)
            if n_cores > 1:
                ar_in = dram_pool.tile([1, 1], F32, tag="ar_in", name="ar_in")
                ar_out = dram_pool.tile([1, 1], F32, tag="ar_out",
                                        name="ar_out")
                nc.sync.dma_start(ar_in[:], s11[:])
                nc.gpsimd.collective_compute(
                    "AllReduce", Alu.add,
                    replica_groups=[list(range(n_cores))],
                    ins=[ar_in.opt()], outs=[ar_out.opt()])
                nc.sync.dma_start(g_sb[:], ar_out[:])
            else:
                loc = dram_pool.tile([1, 1], F32, tag="loc", name="loc")
                nc.sync.dma_start(loc[:], s11[:])
                nc.sync.dma_start(g_sb[:], loc[:])

        # ---- sum-of-squares in MAXCHUNK passes, during the AR wait:
        # blocks 0..N_SQ_DVE-1 on DVE (scalar_tensor_tensor + accum), the
        # rest on ACT (Square + accum).
        nq = T_ // MAXCHUNK
        for b in range(n_blocks):
            ret_t = ret_tiles[b]
            ssp = ss_part_tiles[b]
            for q in range(nq):
                cols = slice(q * MAXCHUNK, (q + 1) * MAXCHUNK)
                if b < N_SQ_DVE:
                    nc.vector.scalar_tensor_tensor(
                        dve_scr[:], ret_t[:, cols], 1.0, ret_t[:, cols],
                        Alu.mult, Alu.mult, accum_out=ssp[:, q:q + 1])
                else:
                    nc.scalar.activation(act_scr[:], ret_t[:, cols],
                                         Act.Square,
                                         accum_out=ssp[:, q:q + 1])
        for b in range(n_blocks):
            nc.vector.tensor_reduce(ss_cat[:, b:b + 1], ss_part_tiles[b][:],
                                    AxL.X, Alu.add)

        # ---- per-row 1/(std+eps): independent of the AllReduce ----
        sum_sq = stat_pool.tile([P, n_blocks], F32)
        nc.vector.tensor_tensor(sum_sq[:], sum_cat[:], sum_cat[:], Alu.mult)
        u = stat_pool.tile([P, n_blocks], F32)
        nc.vector.scalar_tensor_tensor(u[:], sum_sq[:], -1.0 / T_, ss_cat[:],
                                       Alu.mult, Alu.add)  # ss - sum^2/T
        stdv = stat_pool.tile([P, n_blocks], F32)
        nc.scalar.activation(stdv[:], u[:], Act.Sqrt, scale=1.0 / (T_ - 1))
        nc.vector.tensor_scalar_add(stdv[:], stdv[:], EPS)
        inv_cat = stat_pool.tile([P, n_blocks], F32)
        nc.vector.reciprocal(inv_cat[:], stdv[:])

        # The AR result comes back as a [1,1] DMA to partition 0, then a
        # ones[1,128] matmul replicates it across partitions in PSUM (~25ns)
        # -- cheaper than a 128-packet partition-broadcast DMA.
        ones_row = stat_pool.tile([1, P], F32)
        nc.vector.memset(ones_row[:], 1.0)
        psum_g = psum_pool.tile([P, 1], F32, tag="psum_g", name="psum_g")
        nc.tensor.matmul(psum_g[:], ones_row[:], g_sb[:], start=True, stop=True)

        negb_cat = stat_pool.tile([P, n_blocks], F32)
        nc.vector.tensor_scalar(negb_cat[:], inv_cat[:], psum_g[:, 0:1],
                                -1.0 / total_elems, Alu.mult, Alu.mult)

        # ---- normalize in place (bf16 4x mode), stream out per half block ----
        half = T_ // 2
        for b in range(n_blocks):
            rows = slice(b * P, (b + 1) * P)
            ret_t = ret_tiles[b]
            for h in range(2):
                cols = slice(h * half, (h + 1) * half)
                nc.vector.tensor_scalar(ret_t[:, cols], ret_t[:, cols],
                                        inv_cat[:, b:b + 1],
                                        negb_cat[:, b:b + 1],
                                        Alu.mult, Alu.add)
                nc.sync.dma_start(out_ap[rows, cols], ret_t[:, cols])


_NC_CACHE = None


def _get_nc():
    global _NC_CACHE
    if _NC_CACHE is None:
        nc = bacc.Bacc("TRN2", target_bir_lowering=False, debug=False,
                       enable_asserts=False, num_devices=N_CORES)
        rew = nc.dram_tensor("rewards", [B_CORE, T], BF16, kind="ExternalInput")
        done = nc.dram_tensor("done_flags", [B_CORE, T], U8, kind="ExternalInput")
        out = nc.dram_tensor("out", [B_CORE, T], BF16, kind="ExternalOutput")
        with tile.TileContext(nc) as tc:
            _build_core_program(tc, out.ap(), rew.ap(), done.ap(),
                                n_cores=N_CORES, total_elems=B_GLOBAL * T)
        nc.compile()
        _NC_CACHE = nc
    return _NC_CACHE


def run_sharded(rewards, done_flags, trace=False, **kwargs):
    """Run the SPMD kernel; returns (full_output, BassKernelResults)."""
    nc = _get_nc()
    rew16 = rewards.astype(ml_dtypes.bfloat16)
    done8 = done_flags.astype(np.uint8)
    in_maps = []
    for c in range(N_CORES):
        rows = slice(c * B_CORE, (c + 1) * B_CORE)
        in_maps.append({
            "rewards": np.ascontiguousarray(rew16[rows]),
            "done_flags": np.ascontiguousarray(done8[rows]),
        })
    res = run_bass_kernel_spmd(nc, in_maps, core_ids=list(range(N_CORES)),
                               trace=trace, **kwargs)
    full = np.concatenate(
        [res.results[c]["out"].astype(np.float32) for c in range(N_CORES)],
        axis=0)
    return full, res


def kernel(rewards, done_flags):
    out, _ = run_sharded(rewards, done_flags, trace=False)
    return out
